# revision 8
# baseline (speedup 1.0000x reference)
"""Trainium2 Bass kernel for the 2-layer GRU greedy decoder (nn_Decoder).

Strategy (8 NeuronCores, SPMD):
  - W_out / b_out sharded row-wise over vocab: 4000 rows/core, padded to 4096,
    kept SBUF-resident as the matmul *moving* operand (f32r = tf32, 1 cyc/row).
  - GRU weights replicated and SBUF-resident (also moving operands).
  - Batch (32) lives in the stationary operand; activations are transposed
    per step via PE transpose.
  - Biases folded in as K=1 ones-row matmuls accumulating into PSUM.
  - Per step each core computes the full GRU + its vocab shard's logits,
    takes per-512-chunk argmax candidates (DVE max/max_index), and all 8
    chunk candidates are AllGather'd; every core then picks the global
    argmax (first-occurrence tie-break via the min-index trick) to get the
    next token, gathers its embedding row by indirect DMA, and continues.
  - Tokens ([32, 64] int32) and final hidden state ([2, 32, 512] f32) are
    written out at the end; all cores produce identical copies.

Modes (K_MODE env): "f32r" (default, tf32 matmuls) or "3pass" (hi/lo bf16
3-pass matmuls, ~4.5e-6 rel error) for the GRU and/or logits:
K_MODE=f32r|3pass|gru3pass (gru3pass: GRU 3-pass, logits f32r).
"""
import os

import numpy as np

import concourse.bacc as bacc
import concourse.bass as bass
import concourse.mybir as mybir
import concourse.tile as tile
from concourse.bass_utils import run_bass_kernel_spmd
from concourse.masks import make_identity

NC = 8
B = 32
H = 512
E = 512
V = 32000
VS = V // NC          # 4000 vocab rows per core
VSP = 4096            # padded shard width
NCH = VSP // 512      # 8 logit chunks per core
F32 = mybir.dt.float32
F32R = mybir.dt.float32r
BF16 = mybir.dt.bfloat16
I32 = mybir.dt.int32
U32 = mybir.dt.uint32
SIG = mybir.ActivationFunctionType.Sigmoid
TANH = mybir.ActivationFunctionType.Tanh
AOP = mybir.AluOpType

MODE = os.environ.get("K_MODE", "3pass")

_cache = {}


def _round_tf32(a):
    """Round-to-nearest f32 -> tf32 (10-bit mantissa) so the PE's f32r
    truncation of pre-rounded values is exact RNE."""
    u = a.astype(np.float32).view(np.uint32)
    u2 = (u + 0x0FFF + ((u >> 13) & 1)) & np.uint32(0xFFFFE000)
    return u2.view(np.float32)


def _to_kT(w):
    """[N, K] weight -> [128, K//128, N] moving-operand layout (w.T chunked)."""
    K = w.shape[1]
    return np.ascontiguousarray(w.T.reshape(K // 128, 128, -1).transpose(1, 0, 2))


def _hi_lo(a):
    import ml_dtypes
    hi = a.astype(ml_dtypes.bfloat16)
    lo = (a - hi.astype(np.float32)).astype(ml_dtypes.bfloat16)
    return hi, lo


def build(T, gru3, log3, nocc=False):
    """Build the Bass graph. gru3/log3: use 3-pass bf16 for GRU / logits."""
    nc = bacc.Bacc("TRN2", target_bir_lowering=False, num_devices=NC)

    gdt = BF16 if gru3 else F32R
    ldt = BF16 if log3 else F32R
    gru_copies = 2 if gru3 else 1   # hi/lo weight tensors
    log_copies = 2 if log3 else 1

    # ---------------- DRAM inputs ----------------
    def din(name, shape, dt):
        return nc.dram_tensor(name, shape, dt, kind="ExternalInput")

    d = {}
    for li in range(2):
        for part in ("x", "h"):
            for ci in range(gru_copies):
                nm = f"w{li}{part}_{ci}"
                d[nm] = din(nm, [128, 4, 3 * H], gdt)
    for ci in range(log_copies):
        d[f"wout_{ci}"] = din(f"wout_{ci}", [128, 4, VSP], ldt)
    d["bias_all"] = din("bias_all", [16, 512], F32R)
    d["emb"] = din("emb", [V, E], F32)
    d["x0T"] = din("x0T", [128, 4, B], F32)
    d["h0T"] = din("h0T", [128, 4, B], F32)
    d["h1T"] = din("h1T", [128, 4, B], F32)
    d["h0raw"] = din("h0raw", [B, H], F32)
    d["h1raw"] = din("h1raw", [B, H], F32)
    d["chunkoff"] = din("chunkoff", [B, NCH], F32)
    if not gru3:
        d["onesr"] = din("onesr", [96, B], F32R)

    out_toks = nc.dram_tensor("out_toks", [B, T], I32, kind="ExternalOutput")
    out_h = nc.dram_tensor("out_h", [2, B, H], F32, kind="ExternalOutput")

    with tile.TileContext(nc) as tc:
        with (
            tc.tile_pool(name="sbw", bufs=1) as sbw,      # resident weights
            tc.tile_pool(name="sba", bufs=1) as sba,      # in-step activations
            tc.tile_pool(name="sbp", bufs=2) as sbp,      # cross-step (ping-pong)
            tc.tile_pool(name="prz", bufs=1, space="PSUM") as prz,
            tc.tile_pool(name="pn", bufs=1, space="PSUM") as pn,
            tc.tile_pool(name="plog", bufs=3, space="PSUM") as plog,
            tc.tile_pool(name="ptr", bufs=1, space="PSUM") as ptr,
            tc.tile_pool(name="dram", bufs=1, space="DRAM") as dr,
        ):
            # ---------------- resident loads ----------------
            W = {}
            for li in range(2):
                for part in ("x", "h"):
                    for ci in range(gru_copies):
                        nm = f"w{li}{part}_{ci}"
                        t_ = sbw.tile([128, 4, 3 * H], gdt, tag=nm)
                        nc.sync.dma_start(t_[:], d[nm][:])
                        W[nm] = t_
            WO = []
            for ci in range(log_copies):
                t_ = sbw.tile([128, 4, VSP], ldt, tag=f"wout_{ci}")
                nc.sync.dma_start(t_[:], d[f"wout_{ci}"][:])
                WO.append(t_)
            bias_t = sbw.tile([16, 512], F32R, tag="bias")
            nc.sync.dma_start(bias_t[:], d["bias_all"][:])
            ones_t = sbw.tile([1, B], F32R, tag="ones")
            nc.sync.dma_start(ones_t[:], d["onesr"][:])
            chunkoff_t = sbw.tile([B, NCH], F32, tag="chunkoff")
            nc.sync.dma_start(chunkoff_t[:], d["chunkoff"][:])
            ident = sbw.tile([128, 128], F32, tag="ident")
            make_identity(nc, ident[:])
            toks_sb = sbw.tile([B, T], I32, tag="toks")

            # initial states
            xT0 = sbw.tile([128, 4, B], F32, tag="xT0")
            nc.sync.dma_start(xT0[:], d["x0T"][:])
            hT0_in = [sbw.tile([128, 4, B], F32, tag=f"hT{li}in") for li in range(2)]
            nc.sync.dma_start(hT0_in[0][:], d["h0T"][:])
            nc.sync.dma_start(hT0_in[1][:], d["h1T"][:])
            hraw_in = [sbw.tile([B, H], F32, tag=f"hraw{li}in") for li in range(2)]
            nc.sync.dma_start(hraw_in[0][:], d["h0raw"][:])
            nc.sync.dma_start(hraw_in[1][:], d["h1raw"][:])

            if gru3:
                def stat_pair(f32ap, tagbase):
                    """f32 [128,4,B] -> (hi, lo) bf16 stationary tiles."""
                    hi = sbp.tile([128, 4, B], BF16, tag=tagbase + "_hi")
                    nc.vector.tensor_copy(hi[:], f32ap)
                    hif = sba.tile([128, 4, B], F32, tag="stat_hif")
                    nc.vector.tensor_copy(hif[:], hi[:])
                    lof = sba.tile([128, 4, B], F32, tag="stat_lof")
                    nc.vector.tensor_tensor(out=lof[:], in0=f32ap, in1=hif[:],
                                            op=AOP.subtract)
                    lo = sbp.tile([128, 4, B], BF16, tag=tagbase + "_lo")
                    nc.vector.tensor_copy(lo[:], lof[:])
                    return hi, lo
            else:
                def stat_one(f32ap, tagbase):
                    s = sbp.tile([128, 4, B], F32R, tag=tagbase)
                    nc.vector.tensor_copy(s[:], f32ap)
                    return s

            def gru_matmuls(psum_ap, xstat, wkey, cols, first):
                """Accumulate sum_k xstat.T @ W[:, k, cols] into psum_ap.
                xstat: list of stationary tiles (1 for f32r, (hi, lo) for 3pass).
                wkey: weight name base; cols: slice of the 3H axis."""
                if gru3:
                    xhi, xlo = xstat
                    whi, wlo = W[wkey + "_0"], W[wkey + "_1"]
                    passes = [(xhi, whi), (xhi, wlo), (xlo, whi)]
                else:
                    passes = [(xstat[0], W[wkey + "_0"])]
                st = first
                for (xs, ws) in passes:
                    for k in range(4):
                        nc.tensor.matmul(psum_ap, lhsT=xs[:, k, :],
                                         rhs=ws[:, k, cols],
                                         start=st, stop=False)
                        st = False

            def bias_mm(psum_ap, row):
                nc.tensor.matmul(psum_ap, lhsT=ones_t[:],
                                 rhs=bias_t[row:row + 1, :],
                                 start=False, stop=True)

            def transpose_group(raw_ap, tagbase, make_pair):
                """[32, 512] f32 -> stationary tile(s) [128, 4, 32]."""
                tr = ptr.tile([128, 128], F32, tag="ptr")
                for k in range(4):
                    nc.tensor.transpose(out=tr[:, k * B:(k + 1) * B],
                                        in_=raw_ap[:, k * 128:(k + 1) * 128],
                                        identity=ident[:B, :B])
                trv = tr[:].rearrange("p (k b) -> p k b", k=4)
                if make_pair:
                    return stat_pair(trv, tagbase)
                return (stat_one(trv, tagbase),)

            # prepare step-0 stationaries from the f32 inputs
            if gru3:
                xT = stat_pair(xT0[:], "xT")
                hT = [stat_pair(hT0_in[li][:], f"hT{li}") for li in range(2)]
            else:
                xT = (stat_one(xT0[:], "xT"),)
                hT = [(stat_one(hT0_in[li][:], f"hT{li}"),) for li in range(2)]
            hraw = [hraw_in[0], hraw_in[1]]

            if log3:
                def log_passes(h1stat):
                    hhi, hlo = h1stat
                    return [(hhi, WO[0]), (hhi, WO[1]), (hlo, WO[0])]
            else:
                def log_passes(h1stat):
                    return [(h1stat[0], WO[0])]

            # ---------------- decode loop ----------------
            for t in range(T):
                # ---- GRU layers
                new_hraw = []
                new_hT = []
                for li in range(2):
                    wx = f"w{li}x"
                    wh = f"w{li}h"
                    xs = xT if li == 0 else new_hT[0]
                    p_rz = prz.tile([B, 1024], F32, tag="prz")
                    p_n = pn.tile([B, 1024], F32, tag="pn")
                    # h parts first (independent of the incoming token) so the
                    # scheduler can hoist them into the previous step's tail,
                    # then biases, then x parts.
                    gru_matmuls(p_rz[:, 0:512], hT[li], wh, slice(0, 512), True)
                    gru_matmuls(p_rz[:, 512:1024], hT[li], wh, slice(512, 1024), True)
                    gru_matmuls(p_n[:, 512:1024], hT[li], wh, slice(1024, 1536), True)
                    bias_mm(p_n[:, 512:1024], 4 * li + 3)
                    gru_matmuls(p_rz[:, 0:512], xs, wx, slice(0, 512), False)
                    bias_mm(p_rz[:, 0:512], 4 * li + 0)
                    gru_matmuls(p_rz[:, 512:1024], xs, wx, slice(512, 1024), False)
                    bias_mm(p_rz[:, 512:1024], 4 * li + 1)
                    gru_matmuls(p_n[:, 0:512], xs, wx, slice(1024, 1536), True)
                    bias_mm(p_n[:, 0:512], 4 * li + 2)

                    # gates
                    rz = sba.tile([B, 1024], F32, tag="rz")
                    nc.scalar.activation(out=rz[:], in_=p_rz[:], func=SIG)
                    t1 = sba.tile([B, 512], F32, tag="t1")
                    nc.vector.tensor_tensor(out=t1[:], in0=rz[:, 0:512],
                                            in1=p_n[:, 512:1024], op=AOP.mult)
                    t2 = sba.tile([B, 512], F32, tag="t2")
                    nc.vector.tensor_tensor(out=t2[:], in0=t1[:],
                                            in1=p_n[:, 0:512], op=AOP.add)
                    n_sb = sba.tile([B, 512], F32, tag="n_sb")
                    nc.scalar.activation(out=n_sb[:], in_=t2[:], func=TANH)
                    dmn = sba.tile([B, 512], F32, tag="dmn")
                    nc.vector.tensor_tensor(out=dmn[:], in0=hraw[li][:],
                                            in1=n_sb[:], op=AOP.subtract)
                    zd = sba.tile([B, 512], F32, tag="zd")
                    nc.vector.tensor_tensor(out=zd[:], in0=dmn[:],
                                            in1=rz[:, 512:1024], op=AOP.mult)
                    hnew = sbp.tile([B, 512], F32, tag=f"hnew{li}")
                    nc.vector.tensor_tensor(out=hnew[:], in0=zd[:],
                                            in1=n_sb[:], op=AOP.add)
                    new_hraw.append(hnew)
                    new_hT.append(transpose_group(hnew[:], f"hT{li}", gru3))

                # ---- logits + per-chunk argmax candidates
                cv8 = sba.tile([B, NCH, 8], F32, tag="cv8")
                ci8 = sba.tile([B, NCH, 8], U32, tag="ci8")
                for j in range(NCH):
                    p_l = plog.tile([B, 512], F32, tag="plog")
                    st = True
                    for (hs, ws) in log_passes(new_hT[1]):
                        for k in range(4):
                            nc.tensor.matmul(p_l[:], lhsT=hs[:, k, :],
                                             rhs=ws[:, k, 512 * j:512 * (j + 1)],
                                             start=st, stop=False)
                            st = False
                    bias_mm(p_l[:], 8 + j)
                    nc.vector.max(out=cv8[:, j, :], in_=p_l[:])
                    nc.vector.max_index(out=ci8[:, j, :], in_max=cv8[:, j, :],
                                        in_values=p_l[:])

                # chunk-local idx -> global vocab idx (f32, exact below 2^24)
                gidx = sba.tile([B, NCH], F32, tag="gidx")
                nc.vector.tensor_copy(gidx[:], ci8[:, :, 0])
                nc.vector.tensor_tensor(out=gidx[:], in0=gidx[:],
                                        in1=chunkoff_t[:], op=AOP.add)

                # ---- exchange: AllGather the 8 chunk candidates of all cores
                cand = sba.tile([B, 2 * NCH], F32, tag="cand")
                nc.vector.tensor_copy(cand[:, 0:NCH], cv8[:, :, 0])
                nc.vector.tensor_copy(cand[:, NCH:2 * NCH], gidx[:])
                cin = dr.tile([B, 2 * NCH], F32, tag="cin")
                cout = dr.tile([NC * B, 2 * NCH], F32, tag="cout")
                nc.sync.dma_start(cin[:], cand[:])
                if nocc:
                    for cc_i in range(NC):
                        nc.sync.dma_start(cout[cc_i * B:(cc_i + 1) * B, :], cand[:])
                else:
                    nc.gpsimd.collective_compute(
                        "AllGather", AOP.bypass,
                        replica_groups=[list(range(NC))],
                        ins=[cin[:].opt()], outs=[cout[:].opt()])
                agv = sba.tile([B, NC * NCH], F32, tag="agv")
                agi = sba.tile([B, NC * NCH], F32, tag="agi")
                cc = cout[:].rearrange("(c b) t -> b c t", b=B)
                nc.sync.dma_start(agv[:], cc[:, :, 0:NCH])
                nc.sync.dma_start(agi[:], cc[:, :, NCH:2 * NCH])

                # ---- global argmax with first-occurrence tie-break
                gmax = sba.tile([B, 1], F32, tag="gmax")
                nc.vector.tensor_reduce(out=gmax[:], in_=agv[:], op=AOP.max,
                                        axis=mybir.AxisListType.X)
                mask = sba.tile([B, NC * NCH], F32, tag="mask")
                nc.vector.tensor_scalar(mask[:], agv[:], gmax[:, 0:1], None,
                                        op0=AOP.is_ge)
                msel = sba.tile([B, NC * NCH], F32, tag="msel")
                nc.vector.tensor_scalar(msel[:], agi[:], 1e7, None,
                                        op0=AOP.subtract)
                nc.vector.tensor_tensor(out=msel[:], in0=msel[:], in1=mask[:],
                                        op=AOP.mult)
                nc.vector.tensor_scalar(msel[:], msel[:], 1e7, None, op0=AOP.add)
                tok_f = sba.tile([B, 1], F32, tag="tok_f")
                nc.vector.tensor_reduce(out=tok_f[:], in_=msel[:], op=AOP.min,
                                        axis=mybir.AxisListType.X)
                tok_i = sbp.tile([B, 1], I32, tag="tok_i")
                nc.vector.tensor_copy(tok_i[:], tok_f[:])
                nc.vector.tensor_copy(toks_sb[:, t:t + 1], tok_i[:])

                # ---- next x: gather + transpose (skip after last step)
                hraw = new_hraw
                hT = new_hT
                if t + 1 < T:
                    x_raw = sba.tile([B, E], F32, tag="x_raw")
                    nc.gpsimd.indirect_dma_start(
                        out=x_raw[:], out_offset=None, in_=d["emb"][:],
                        in_offset=bass.IndirectOffsetOnAxis(ap=tok_i[:, 0:1], axis=0))
                    xT = transpose_group(x_raw[:], "xT", gru3)

            # ---------------- outputs ----------------
            nc.sync.dma_start(out_toks[:], toks_sb[:])
            nc.sync.dma_start(out_h[0], hraw[0][:])
            nc.sync.dma_start(out_h[1], hraw[1][:])

    nc.compile()
    return nc


def _prep_inputs(h_0, emb, W_ih0, W_hh0, b_ih0, b_hh0, W_ih1, W_hh1, b_ih1,
                 b_hh1, W_out, b_out, gru3, log3):
    SOS = 1
    rnd = (lambda a: a) if True else None
    gw = {}
    for li, (wx, wh) in enumerate([(W_ih0, W_hh0), (W_ih1, W_hh1)]):
        for part, w in (("x", wx), ("h", wh)):
            kT = _to_kT(w)          # [128, 4, 1536]
            if gru3:
                hi, lo = _hi_lo(kT)
                gw[f"w{li}{part}_0"] = hi
                gw[f"w{li}{part}_1"] = lo
            else:
                gw[f"w{li}{part}_0"] = _round_tf32(kT)

    x0 = np.repeat(emb[SOS][None, :], B, axis=0)        # [32, 512]
    common = dict(
        emb=np.ascontiguousarray(emb),
        x0T=np.ascontiguousarray(x0.T.reshape(4, 128, B).transpose(1, 0, 2)),
        h0T=np.ascontiguousarray(h_0[0].T.reshape(4, 128, B).transpose(1, 0, 2)),
        h1T=np.ascontiguousarray(h_0[1].T.reshape(4, 128, B).transpose(1, 0, 2)),
        h0raw=np.ascontiguousarray(h_0[0]),
        h1raw=np.ascontiguousarray(h_0[1]),
        **gw,
    )

    brz0 = (b_ih0 + b_hh0)[0:1024]
    brz1 = (b_ih1 + b_hh1)[0:1024]
    in_maps = []
    for c in range(NC):
        wsh = W_out[c * VS:(c + 1) * VS]
        wpad = np.zeros((VSP, H), np.float32)
        wpad[:VS] = wsh
        kT = _to_kT(wpad)                                # [128, 4, 4096]
        wo = {}
        if log3:
            hi, lo = _hi_lo(kT)
            wo["wout_0"], wo["wout_1"] = hi, lo
        else:
            wo["wout_0"] = _round_tf32(kT)
        bpad = np.full((VSP,), -1e30, np.float32)
        bpad[:VS] = b_out[c * VS:(c + 1) * VS]
        bias_all = np.zeros((16, 512), np.float32)
        bias_all[0:2] = brz0.reshape(2, 512)
        bias_all[2] = b_ih0[1024:1536]
        bias_all[3] = b_hh0[1024:1536]
        bias_all[4:6] = brz1.reshape(2, 512)
        bias_all[6] = b_ih1[1024:1536]
        bias_all[7] = b_hh1[1024:1536]
        bias_all[8:16] = bpad.reshape(8, 512)
        chunkoff = np.zeros((B, NCH), np.float32)
        chunkoff[:] = (c * VS + 512 * np.arange(NCH))[None, :]
        in_maps.append(dict(common, bias_all=_round_tf32(bias_all),
                            chunkoff=chunkoff, **wo))
    return in_maps


def kernel(h_0, emb, W_ih0, W_hh0, b_ih0, b_hh0, W_ih1, W_hh1, b_ih1, b_hh1,
           W_out, b_out, max_n, _trace=False):
    T = int(max_n)
    gru3 = MODE in ("3pass", "gru3pass")
    log3 = MODE == "3pass"
    args = tuple(np.asarray(a) for a in
                 (h_0, emb, W_ih0, W_hh0, b_ih0, b_hh0, W_ih1, W_hh1,
                  b_ih1, b_hh1, W_out, b_out))
    in_maps = _prep_inputs(*[np.asarray(a, np.float32) for a in args],
                           gru3=gru3, log3=log3)
    key = (T, gru3, log3)
    if key not in _cache:
        _cache[key] = build(T, gru3, log3)
    nc = _cache[key]
    res = run_bass_kernel_spmd(nc, in_maps, core_ids=list(range(NC)),
                               trace=_trace)
    r0 = res.results[0]
    toks = r0["out_toks"].astype(np.int32)
    h_t = r0["out_h"].astype(np.float32)
    if _trace:
        kernel.last_exec_ns = res.exec_time_ns
    return toks, h_t


# revision 10
# speedup vs baseline: 90.3194x; 90.3194x over previous
"""Trainium2 Bass kernel for the 2-layer GRU greedy decoder (nn_Decoder).

Strategy (8 NeuronCores, SPMD):
  - W_out / b_out sharded row-wise over vocab: 4000 rows/core, padded to 4096,
    kept SBUF-resident as the matmul *moving* operand (f32r = tf32, 1 cyc/row).
  - GRU weights replicated and SBUF-resident (also moving operands).
  - Batch (32) lives in the stationary operand; activations are transposed
    per step via PE transpose.
  - Biases folded in as K=1 ones-row matmuls accumulating into PSUM.
  - Per step each core computes the full GRU + its vocab shard's logits,
    takes per-512-chunk argmax candidates (DVE max/max_index), and all 8
    chunk candidates are AllGather'd; every core then picks the global
    argmax (first-occurrence tie-break via the min-index trick) to get the
    next token, gathers its embedding row by indirect DMA, and continues.
  - Tokens ([32, 64] int32) and final hidden state ([2, 32, 512] f32) are
    written out at the end; all cores produce identical copies.

Modes (K_MODE env): "f32r" (default, tf32 matmuls) or "3pass" (hi/lo bf16
3-pass matmuls, ~4.5e-6 rel error) for the GRU and/or logits:
K_MODE=f32r|3pass|gru3pass (gru3pass: GRU 3-pass, logits f32r).
"""
import os

import numpy as np

import concourse.bacc as bacc
import concourse.bass as bass
import concourse.mybir as mybir
import concourse.tile as tile
from concourse.bass_utils import run_bass_kernel_spmd
from concourse.masks import make_identity

NC = 8
B = 32
H = 512
E = 512
V = 32000
VS = V // NC          # 4000 vocab rows per core
VSP = 4096            # padded shard width
NCH = VSP // 512      # 8 logit chunks per core
F32 = mybir.dt.float32
F32R = mybir.dt.float32r
BF16 = mybir.dt.bfloat16
I32 = mybir.dt.int32
U32 = mybir.dt.uint32
SIG = mybir.ActivationFunctionType.Sigmoid
TANH = mybir.ActivationFunctionType.Tanh
AOP = mybir.AluOpType

MODE = os.environ.get("K_MODE", "3pass")

_cache = {}


def _round_tf32(a):
    """Round-to-nearest f32 -> tf32 (10-bit mantissa) so the PE's f32r
    truncation of pre-rounded values is exact RNE."""
    u = a.astype(np.float32).view(np.uint32)
    u2 = (u + 0x0FFF + ((u >> 13) & 1)) & np.uint32(0xFFFFE000)
    return u2.view(np.float32)


def _to_kT(w):
    """[N, K] weight -> [128, K//128, N] moving-operand layout (w.T chunked)."""
    K = w.shape[1]
    return np.ascontiguousarray(w.T.reshape(K // 128, 128, -1).transpose(1, 0, 2))


def _hi_lo(a):
    import ml_dtypes
    hi = a.astype(ml_dtypes.bfloat16)
    lo = (a - hi.astype(np.float32)).astype(ml_dtypes.bfloat16)
    return hi, lo


def build(T, gru3, log3, nocc=False):
    """Build the Bass graph. gru3/log3: use 3-pass bf16 for GRU / logits."""
    nc = bacc.Bacc("TRN2", target_bir_lowering=False, num_devices=NC)

    gdt = BF16 if gru3 else F32R
    ldt = BF16 if log3 else F32R
    gru_copies = 2 if gru3 else 1   # hi/lo weight tensors
    log_copies = 2 if log3 else 1

    # ---------------- DRAM inputs ----------------
    def din(name, shape, dt):
        return nc.dram_tensor(name, shape, dt, kind="ExternalInput")

    d = {}
    for li in range(2):
        for part in ("x", "h"):
            for ci in range(gru_copies):
                nm = f"w{li}{part}_{ci}"
                d[nm] = din(nm, [128, 4, 3 * H], gdt)
    for ci in range(log_copies):
        d[f"wout_{ci}"] = din(f"wout_{ci}", [128, 4, VSP], ldt)
    d["bias_all"] = din("bias_all", [16, 512], F32R)
    d["emb"] = din("emb", [V, E], F32)
    d["x0T"] = din("x0T", [128, 4, B], F32)
    d["h0T"] = din("h0T", [128, 4, B], F32)
    d["h1T"] = din("h1T", [128, 4, B], F32)
    d["h0raw"] = din("h0raw", [B, H], F32)
    d["h1raw"] = din("h1raw", [B, H], F32)
    d["chunkoff"] = din("chunkoff", [B, NCH], F32)
    if not gru3:
        d["onesr"] = din("onesr", [96, B], F32R)

    out_toks = nc.dram_tensor("out_toks", [B, T], I32, kind="ExternalOutput")
    out_h = nc.dram_tensor("out_h", [2, B, H], F32, kind="ExternalOutput")

    with tile.TileContext(nc) as tc:
        with (
            tc.tile_pool(name="sbw", bufs=1) as sbw,      # resident weights
            tc.tile_pool(name="sba", bufs=1) as sba,      # in-step activations
            tc.tile_pool(name="sbp", bufs=2) as sbp,      # cross-step (ping-pong)
            tc.tile_pool(name="prz", bufs=1, space="PSUM") as prz,
            tc.tile_pool(name="pn", bufs=1, space="PSUM") as pn,
            tc.tile_pool(name="plog", bufs=3, space="PSUM") as plog,
            tc.tile_pool(name="ptr", bufs=1, space="PSUM") as ptr,
            tc.tile_pool(name="dram", bufs=1, space="DRAM") as dr,
        ):
            # ---------------- resident loads ----------------
            W = {}
            for li in range(2):
                for part in ("x", "h"):
                    for ci in range(gru_copies):
                        nm = f"w{li}{part}_{ci}"
                        t_ = sbw.tile([128, 4, 3 * H], gdt, tag=nm)
                        nc.sync.dma_start(t_[:], d[nm][:])
                        W[nm] = t_
            WO = []
            for ci in range(log_copies):
                t_ = sbw.tile([128, 4, VSP], ldt, tag=f"wout_{ci}")
                nc.sync.dma_start(t_[:], d[f"wout_{ci}"][:])
                WO.append(t_)
            bias_t = sbw.tile([16, 512], F32R, tag="bias")
            nc.sync.dma_start(bias_t[:], d["bias_all"][:])
            ones_t = sbw.tile([1, B], F32R, tag="ones")
            nc.sync.dma_start(ones_t[:], d["onesr"][:])
            chunkoff_t = sbw.tile([B, NCH], F32, tag="chunkoff")
            nc.sync.dma_start(chunkoff_t[:], d["chunkoff"][:])
            ident = sbw.tile([128, 128], F32, tag="ident")
            make_identity(nc, ident[:])
            toks_sb = sbw.tile([B, T], I32, tag="toks")

            # initial states
            xT0 = sbw.tile([128, 4, B], F32, tag="xT0")
            nc.sync.dma_start(xT0[:], d["x0T"][:])
            hT0_in = [sbw.tile([128, 4, B], F32, tag=f"hT{li}in") for li in range(2)]
            nc.sync.dma_start(hT0_in[0][:], d["h0T"][:])
            nc.sync.dma_start(hT0_in[1][:], d["h1T"][:])
            hraw_in = [sbw.tile([B, H], F32, tag=f"hraw{li}in") for li in range(2)]
            nc.sync.dma_start(hraw_in[0][:], d["h0raw"][:])
            nc.sync.dma_start(hraw_in[1][:], d["h1raw"][:])

            if gru3:
                def stat_pair(f32ap, tagbase):
                    """f32 [128,4,B] -> (hi, lo) bf16 stationary tiles."""
                    hi = sbp.tile([128, 4, B], BF16, tag=tagbase + "_hi")
                    nc.vector.tensor_copy(hi[:], f32ap)
                    hif = sba.tile([128, 4, B], F32, tag="stat_hif")
                    nc.vector.tensor_copy(hif[:], hi[:])
                    lof = sba.tile([128, 4, B], F32, tag="stat_lof")
                    nc.vector.tensor_tensor(out=lof[:], in0=f32ap, in1=hif[:],
                                            op=AOP.subtract)
                    lo = sbp.tile([128, 4, B], BF16, tag=tagbase + "_lo")
                    nc.vector.tensor_copy(lo[:], lof[:])
                    return hi, lo
            else:
                def stat_one(f32ap, tagbase):
                    s = sbp.tile([128, 4, B], F32R, tag=tagbase)
                    nc.vector.tensor_copy(s[:], f32ap)
                    return s

            def gru_matmuls(psum_ap, xstat, wkey, cols, first):
                """Accumulate sum_k xstat.T @ W[:, k, cols] into psum_ap.
                xstat: list of stationary tiles (1 for f32r, (hi, lo) for 3pass).
                wkey: weight name base; cols: slice of the 3H axis."""
                if gru3:
                    xhi, xlo = xstat
                    whi, wlo = W[wkey + "_0"], W[wkey + "_1"]
                    passes = [(xhi, whi), (xhi, wlo), (xlo, whi)]
                else:
                    passes = [(xstat[0], W[wkey + "_0"])]
                st = first
                for (xs, ws) in passes:
                    for k in range(4):
                        nc.tensor.matmul(psum_ap, lhsT=xs[:, k, :],
                                         rhs=ws[:, k, cols],
                                         start=st, stop=False)
                        st = False

            def bias_mm(psum_ap, row):
                nc.tensor.matmul(psum_ap, lhsT=ones_t[:],
                                 rhs=bias_t[row:row + 1, :],
                                 start=False, stop=True)

            def transpose_group(raw_ap, tagbase, make_pair):
                """[32, 512] f32 -> stationary tile(s) [128, 4, 32]."""
                tr = ptr.tile([128, 128], F32, tag="ptr")
                for k in range(4):
                    nc.tensor.transpose(out=tr[:, k * B:(k + 1) * B],
                                        in_=raw_ap[:, k * 128:(k + 1) * 128],
                                        identity=ident[:B, :B])
                trv = tr[:].rearrange("p (k b) -> p k b", k=4)
                if make_pair:
                    return stat_pair(trv, tagbase)
                return (stat_one(trv, tagbase),)

            # prepare step-0 stationaries from the f32 inputs
            if gru3:
                xT = stat_pair(xT0[:], "xT")
                hT = [stat_pair(hT0_in[li][:], f"hT{li}") for li in range(2)]
            else:
                xT = (stat_one(xT0[:], "xT"),)
                hT = [(stat_one(hT0_in[li][:], f"hT{li}"),) for li in range(2)]
            hraw = [hraw_in[0], hraw_in[1]]

            if log3:
                def log_passes(h1stat):
                    hhi, hlo = h1stat
                    return [(hhi, WO[0]), (hhi, WO[1]), (hlo, WO[0])]
            else:
                def log_passes(h1stat):
                    return [(h1stat[0], WO[0])]

            # ---------------- decode loop ----------------
            for t in range(T):
                # ---- GRU layers
                new_hraw = []
                new_hT = []
                for li in range(2):
                    wx = f"w{li}x"
                    wh = f"w{li}h"
                    xs = xT if li == 0 else new_hT[0]
                    p_rz = prz.tile([B, 1024], F32, tag="prz")
                    p_n = pn.tile([B, 1024], F32, tag="pn")
                    # h parts first (independent of the incoming token) so the
                    # scheduler can hoist them into the previous step's tail,
                    # then biases, then x parts.
                    gru_matmuls(p_rz[:, 0:512], hT[li], wh, slice(0, 512), True)
                    gru_matmuls(p_rz[:, 512:1024], hT[li], wh, slice(512, 1024), True)
                    gru_matmuls(p_n[:, 512:1024], hT[li], wh, slice(1024, 1536), True)
                    bias_mm(p_n[:, 512:1024], 4 * li + 3)
                    gru_matmuls(p_rz[:, 0:512], xs, wx, slice(0, 512), False)
                    bias_mm(p_rz[:, 0:512], 4 * li + 0)
                    gru_matmuls(p_rz[:, 512:1024], xs, wx, slice(512, 1024), False)
                    bias_mm(p_rz[:, 512:1024], 4 * li + 1)
                    gru_matmuls(p_n[:, 0:512], xs, wx, slice(1024, 1536), True)
                    bias_mm(p_n[:, 0:512], 4 * li + 2)

                    # gates
                    rz = sba.tile([B, 1024], F32, tag="rz")
                    nc.scalar.activation(out=rz[:], in_=p_rz[:], func=SIG)
                    t1 = sba.tile([B, 512], F32, tag="t1")
                    nc.vector.tensor_tensor(out=t1[:], in0=rz[:, 0:512],
                                            in1=p_n[:, 512:1024], op=AOP.mult)
                    t2 = sba.tile([B, 512], F32, tag="t2")
                    nc.vector.tensor_tensor(out=t2[:], in0=t1[:],
                                            in1=p_n[:, 0:512], op=AOP.add)
                    n_sb = sba.tile([B, 512], F32, tag="n_sb")
                    nc.scalar.activation(out=n_sb[:], in_=t2[:], func=TANH)
                    dmn = sba.tile([B, 512], F32, tag="dmn")
                    nc.vector.tensor_tensor(out=dmn[:], in0=hraw[li][:],
                                            in1=n_sb[:], op=AOP.subtract)
                    zd = sba.tile([B, 512], F32, tag="zd")
                    nc.vector.tensor_tensor(out=zd[:], in0=dmn[:],
                                            in1=rz[:, 512:1024], op=AOP.mult)
                    hnew = sbp.tile([B, 512], F32, tag=f"hnew{li}")
                    nc.vector.tensor_tensor(out=hnew[:], in0=zd[:],
                                            in1=n_sb[:], op=AOP.add)
                    new_hraw.append(hnew)
                    new_hT.append(transpose_group(hnew[:], f"hT{li}", gru3))

                # ---- logits + per-chunk argmax candidates
                cv8 = sba.tile([B, NCH, 8], F32, tag="cv8")
                ci8 = sba.tile([B, NCH, 8], U32, tag="ci8")
                for j in range(NCH):
                    p_l = plog.tile([B, 512], F32, tag="plog")
                    st = True
                    for (hs, ws) in log_passes(new_hT[1]):
                        for k in range(4):
                            nc.tensor.matmul(p_l[:], lhsT=hs[:, k, :],
                                             rhs=ws[:, k, 512 * j:512 * (j + 1)],
                                             start=st, stop=False)
                            st = False
                    bias_mm(p_l[:], 8 + j)
                    nc.vector.max(out=cv8[:, j, :], in_=p_l[:])
                    nc.vector.max_index(out=ci8[:, j, :], in_max=cv8[:, j, :],
                                        in_values=p_l[:])

                # chunk-local idx -> global vocab idx (f32, exact below 2^24)
                gidx = sba.tile([B, NCH], F32, tag="gidx")
                nc.vector.tensor_copy(gidx[:], ci8[:, :, 0])
                nc.vector.tensor_tensor(out=gidx[:], in0=gidx[:],
                                        in1=chunkoff_t[:], op=AOP.add)

                # ---- exchange: AllGather the 8 chunk candidates of all cores
                cand = sba.tile([B, 2 * NCH], F32, tag="cand")
                nc.vector.tensor_copy(cand[:, 0:NCH], cv8[:, :, 0])
                nc.vector.tensor_copy(cand[:, NCH:2 * NCH], gidx[:])
                cin = dr.tile([B, 2 * NCH], F32, tag="cin")
                cout = dr.tile([NC * B, 2 * NCH], F32, tag="cout")
                nc.sync.dma_start(cin[:], cand[:])
                if nocc:
                    for cc_i in range(NC):
                        nc.sync.dma_start(cout[cc_i * B:(cc_i + 1) * B, :], cand[:])
                else:
                    nc.gpsimd.collective_compute(
                        "AllGather", AOP.bypass,
                        replica_groups=[list(range(NC))],
                        ins=[cin[:].opt()], outs=[cout[:].opt()])
                agv = sba.tile([B, NC * NCH], F32, tag="agv")
                agi = sba.tile([B, NC * NCH], F32, tag="agi")
                cc = cout[:].rearrange("(c b) t -> b c t", b=B)
                nc.sync.dma_start(agv[:], cc[:, :, 0:NCH])
                nc.sync.dma_start(agi[:], cc[:, :, NCH:2 * NCH])

                # ---- global argmax with first-occurrence tie-break
                gmax = sba.tile([B, 1], F32, tag="gmax")
                nc.vector.tensor_reduce(out=gmax[:], in_=agv[:], op=AOP.max,
                                        axis=mybir.AxisListType.X)
                mask = sba.tile([B, NC * NCH], F32, tag="mask")
                nc.vector.tensor_scalar(mask[:], agv[:], gmax[:, 0:1], None,
                                        op0=AOP.is_ge)
                msel = sba.tile([B, NC * NCH], F32, tag="msel")
                nc.vector.tensor_scalar(msel[:], agi[:], 1e7, None,
                                        op0=AOP.subtract)
                nc.vector.tensor_tensor(out=msel[:], in0=msel[:], in1=mask[:],
                                        op=AOP.mult)
                nc.vector.tensor_scalar(msel[:], msel[:], 1e7, None, op0=AOP.add)
                tok_f = sba.tile([B, 1], F32, tag="tok_f")
                nc.vector.tensor_reduce(out=tok_f[:], in_=msel[:], op=AOP.min,
                                        axis=mybir.AxisListType.X)
                tok_i = sbp.tile([B, 1], I32, tag="tok_i")
                nc.vector.tensor_copy(tok_i[:], tok_f[:])
                nc.vector.tensor_copy(toks_sb[:, t:t + 1], tok_i[:])

                # ---- next x: gather + transpose (skip after last step)
                hraw = new_hraw
                hT = new_hT
                if t + 1 < T:
                    x_raw = sba.tile([B, E], F32, tag="x_raw")
                    nc.gpsimd.indirect_dma_start(
                        out=x_raw[:], out_offset=None, in_=d["emb"][:],
                        in_offset=bass.IndirectOffsetOnAxis(ap=tok_i[:, 0:1], axis=0))
                    xT = transpose_group(x_raw[:], "xT", gru3)

            # ---------------- outputs ----------------
            nc.sync.dma_start(out_toks[:], toks_sb[:])
            nc.sync.dma_start(out_h[0], hraw[0][:])
            nc.sync.dma_start(out_h[1], hraw[1][:])

    nc.compile()
    return nc


def _prep_inputs(h_0, emb, W_ih0, W_hh0, b_ih0, b_hh0, W_ih1, W_hh1, b_ih1,
                 b_hh1, W_out, b_out, gru3, log3):
    SOS = 1
    rnd = (lambda a: a) if True else None
    gw = {}
    for li, (wx, wh) in enumerate([(W_ih0, W_hh0), (W_ih1, W_hh1)]):
        for part, w in (("x", wx), ("h", wh)):
            kT = _to_kT(w)          # [128, 4, 1536]
            if gru3:
                hi, lo = _hi_lo(kT)
                gw[f"w{li}{part}_0"] = hi
                gw[f"w{li}{part}_1"] = lo
            else:
                gw[f"w{li}{part}_0"] = _round_tf32(kT)

    x0 = np.repeat(emb[SOS][None, :], B, axis=0)        # [32, 512]
    common = dict(
        emb=np.ascontiguousarray(emb),
        x0T=np.ascontiguousarray(x0.T.reshape(4, 128, B).transpose(1, 0, 2)),
        h0T=np.ascontiguousarray(h_0[0].T.reshape(4, 128, B).transpose(1, 0, 2)),
        h1T=np.ascontiguousarray(h_0[1].T.reshape(4, 128, B).transpose(1, 0, 2)),
        h0raw=np.ascontiguousarray(h_0[0]),
        h1raw=np.ascontiguousarray(h_0[1]),
        **gw,
    )

    brz0 = (b_ih0 + b_hh0)[0:1024]
    brz1 = (b_ih1 + b_hh1)[0:1024]
    in_maps = []
    for c in range(NC):
        wsh = W_out[c * VS:(c + 1) * VS]
        wpad = np.zeros((VSP, H), np.float32)
        wpad[:VS] = wsh
        kT = _to_kT(wpad)                                # [128, 4, 4096]
        wo = {}
        if log3:
            hi, lo = _hi_lo(kT)
            wo["wout_0"], wo["wout_1"] = hi, lo
        else:
            wo["wout_0"] = _round_tf32(kT)
        bpad = np.full((VSP,), -1e30, np.float32)
        bpad[:VS] = b_out[c * VS:(c + 1) * VS]
        bias_all = np.zeros((16, 512), np.float32)
        bias_all[0:2] = brz0.reshape(2, 512)
        bias_all[2] = b_ih0[1024:1536]
        bias_all[3] = b_hh0[1024:1536]
        bias_all[4:6] = brz1.reshape(2, 512)
        bias_all[6] = b_ih1[1024:1536]
        bias_all[7] = b_hh1[1024:1536]
        bias_all[8:16] = bpad.reshape(8, 512)
        chunkoff = np.zeros((B, NCH), np.float32)
        chunkoff[:] = (c * VS + 512 * np.arange(NCH))[None, :]
        in_maps.append(dict(common, bias_all=_round_tf32(bias_all),
                            chunkoff=chunkoff, **wo))
    return in_maps


def kernel(h_0, emb, W_ih0, W_hh0, b_ih0, b_hh0, W_ih1, W_hh1, b_ih1, b_hh1,
           W_out, b_out, max_n, _trace=False):
    T = int(max_n)
    gru3 = MODE in ("3pass", "gru3pass")
    log3 = MODE == "3pass"
    args = tuple(np.asarray(a) for a in
                 (h_0, emb, W_ih0, W_hh0, b_ih0, b_hh0, W_ih1, W_hh1,
                  b_ih1, b_hh1, W_out, b_out))
    in_maps = _prep_inputs(*[np.asarray(a, np.float32) for a in args],
                           gru3=gru3, log3=log3)
    key = (T, gru3, log3)
    if key not in _cache:
        _cache[key] = build(T, gru3, log3)
    nc = _cache[key]
    res = run_bass_kernel_spmd(nc, in_maps, core_ids=list(range(NC)),
                               trace=_trace)
    r0 = res.results[0]
    toks = r0["out_toks"].astype(np.int32)
    h_t = r0["out_h"].astype(np.float32)
    if _trace:
        kernel.last_exec_ns = res.exec_time_ns
    return toks, h_t


# revision 11
# speedup vs baseline: 99.1252x; 1.0975x over previous
"""Trainium2 Bass kernel for the 2-layer GRU greedy decoder (nn_Decoder).

Strategy (8 NeuronCores, SPMD):
  - W_out / b_out sharded row-wise over vocab: 4000 rows/core, padded to 4096,
    kept SBUF-resident as the matmul *moving* operand (f32r = tf32, 1 cyc/row).
  - GRU weights replicated and SBUF-resident (also moving operands).
  - Batch (32) lives in the stationary operand; activations are transposed
    per step via PE transpose.
  - Biases folded in as K=1 ones-row matmuls accumulating into PSUM.
  - Per step each core computes the full GRU + its vocab shard's logits,
    takes per-512-chunk argmax candidates (DVE max/max_index), and all 8
    chunk candidates are AllGather'd; every core then picks the global
    argmax (first-occurrence tie-break via the min-index trick) to get the
    next token, gathers its embedding row by indirect DMA, and continues.
  - Tokens ([32, 64] int32) and final hidden state ([2, 32, 512] f32) are
    written out at the end; all cores produce identical copies.

Modes (K_MODE env): "f32r" (default, tf32 matmuls) or "3pass" (hi/lo bf16
3-pass matmuls, ~4.5e-6 rel error) for the GRU and/or logits:
K_MODE=f32r|3pass|gru3pass (gru3pass: GRU 3-pass, logits f32r).
"""
import os

import numpy as np

import concourse.bacc as bacc
import concourse.bass as bass
import concourse.mybir as mybir
import concourse.tile as tile
from concourse.bass_utils import run_bass_kernel_spmd
from concourse.masks import make_identity

NC = 8
B = 32
H = 512
E = 512
V = 32000
VS = V // NC          # 4000 vocab rows per core
VSP = 4096            # padded shard width
NCH = VSP // 512      # 8 logit chunks per core
F32 = mybir.dt.float32
F32R = mybir.dt.float32r
BF16 = mybir.dt.bfloat16
I32 = mybir.dt.int32
U32 = mybir.dt.uint32
SIG = mybir.ActivationFunctionType.Sigmoid
TANH = mybir.ActivationFunctionType.Tanh
AOP = mybir.AluOpType

MODE = os.environ.get("K_MODE", "3pass")

_cache = {}


def _round_tf32(a):
    """Round-to-nearest f32 -> tf32 (10-bit mantissa) so the PE's f32r
    truncation of pre-rounded values is exact RNE."""
    u = a.astype(np.float32).view(np.uint32)
    u2 = (u + 0x0FFF + ((u >> 13) & 1)) & np.uint32(0xFFFFE000)
    return u2.view(np.float32)


def _to_kT(w):
    """[N, K] weight -> [128, K//128, N] moving-operand layout (w.T chunked)."""
    K = w.shape[1]
    return np.ascontiguousarray(w.T.reshape(K // 128, 128, -1).transpose(1, 0, 2))


def _hi_lo(a):
    import ml_dtypes
    hi = a.astype(ml_dtypes.bfloat16)
    lo = (a - hi.astype(np.float32)).astype(ml_dtypes.bfloat16)
    return hi, lo


def build(T, gru3, log3, nocc=False):
    """Build the Bass graph. gru3/log3: use 3-pass bf16 for GRU / logits."""
    nc = bacc.Bacc("TRN2", target_bir_lowering=False, num_devices=NC)

    gdt = BF16 if gru3 else F32R
    ldt = BF16 if log3 else F32R
    gru_copies = 2 if gru3 else 1   # hi/lo weight tensors
    log_copies = 2 if log3 else 1

    # ---------------- DRAM inputs ----------------
    def din(name, shape, dt):
        return nc.dram_tensor(name, shape, dt, kind="ExternalInput")

    d = {}
    for li in range(2):
        for part in ("x", "h"):
            for ci in range(gru_copies):
                nm = f"w{li}{part}_{ci}"
                d[nm] = din(nm, [128, 4, 3 * H], gdt)
    for ci in range(log_copies):
        d[f"wout_{ci}"] = din(f"wout_{ci}", [128, 4, VSP], ldt)
    d["bias_all"] = din("bias_all", [16, 512], F32R)
    d["emb"] = din("emb", [V, E], F32)
    d["x0T"] = din("x0T", [128, 4, B], F32)
    d["h0T"] = din("h0T", [128, 4, B], F32)
    d["h1T"] = din("h1T", [128, 4, B], F32)
    d["h0raw"] = din("h0raw", [B, H], F32)
    d["h1raw"] = din("h1raw", [B, H], F32)
    d["chunkoff"] = din("chunkoff", [B, NCH], F32)
    if not gru3:
        d["onesr"] = din("onesr", [96, B], F32R)

    out_toks = nc.dram_tensor("out_toks", [B, T], I32, kind="ExternalOutput")
    out_h = nc.dram_tensor("out_h", [2, B, H], F32, kind="ExternalOutput")

    with tile.TileContext(nc) as tc:
        with (
            tc.tile_pool(name="sbw", bufs=1) as sbw,      # resident weights
            tc.tile_pool(name="sba", bufs=1) as sba,      # in-step activations
            tc.tile_pool(name="sbp", bufs=2) as sbp,      # cross-step (ping-pong)
            tc.tile_pool(name="prz", bufs=1, space="PSUM") as prz,
            tc.tile_pool(name="pn", bufs=1, space="PSUM") as pn,
            tc.tile_pool(name="plog", bufs=3, space="PSUM") as plog,
            tc.tile_pool(name="ptr", bufs=1, space="PSUM") as ptr,
            tc.tile_pool(name="dram", bufs=1, space="DRAM") as dr,
        ):
            # ---------------- resident loads ----------------
            W = {}
            for li in range(2):
                for part in ("x", "h"):
                    for ci in range(gru_copies):
                        nm = f"w{li}{part}_{ci}"
                        t_ = sbw.tile([128, 4, 3 * H], gdt, tag=nm)
                        nc.sync.dma_start(t_[:], d[nm][:])
                        W[nm] = t_
            WO = []
            for ci in range(log_copies):
                t_ = sbw.tile([128, 4, VSP], ldt, tag=f"wout_{ci}")
                nc.sync.dma_start(t_[:], d[f"wout_{ci}"][:])
                WO.append(t_)
            bias_t = sbw.tile([16, 512], F32R, tag="bias")
            nc.sync.dma_start(bias_t[:], d["bias_all"][:])
            ones_t = sbw.tile([1, B], F32R, tag="ones")
            nc.sync.dma_start(ones_t[:], d["onesr"][:])
            chunkoff_t = sbw.tile([B, NCH], F32, tag="chunkoff")
            nc.sync.dma_start(chunkoff_t[:], d["chunkoff"][:])
            ident = sbw.tile([128, 128], F32, tag="ident")
            make_identity(nc, ident[:])
            toks_sb = sbw.tile([B, T], I32, tag="toks")

            # initial states
            xT0 = sbw.tile([128, 4, B], F32, tag="xT0")
            nc.sync.dma_start(xT0[:], d["x0T"][:])
            hT0_in = [sbw.tile([128, 4, B], F32, tag=f"hT{li}in") for li in range(2)]
            nc.sync.dma_start(hT0_in[0][:], d["h0T"][:])
            nc.sync.dma_start(hT0_in[1][:], d["h1T"][:])
            hraw_in = [sbw.tile([B, H], F32, tag=f"hraw{li}in") for li in range(2)]
            nc.sync.dma_start(hraw_in[0][:], d["h0raw"][:])
            nc.sync.dma_start(hraw_in[1][:], d["h1raw"][:])

            if gru3:
                def stat_pair(f32ap, tagbase):
                    """f32 [128,4,B] -> (hi, lo) bf16 stationary tiles."""
                    hi = sbp.tile([128, 4, B], BF16, tag=tagbase + "_hi")
                    nc.vector.tensor_copy(hi[:], f32ap)
                    hif = sba.tile([128, 4, B], F32, tag="stat_hif")
                    nc.vector.tensor_copy(hif[:], hi[:])
                    lof = sba.tile([128, 4, B], F32, tag="stat_lof")
                    nc.vector.tensor_tensor(out=lof[:], in0=f32ap, in1=hif[:],
                                            op=AOP.subtract)
                    lo = sbp.tile([128, 4, B], BF16, tag=tagbase + "_lo")
                    nc.vector.tensor_copy(lo[:], lof[:])
                    return hi, lo
            else:
                def stat_one(f32ap, tagbase):
                    s = sbp.tile([128, 4, B], F32R, tag=tagbase)
                    nc.vector.tensor_copy(s[:], f32ap)
                    return s

            def gru_matmuls(psum_ap, xstat, wkey, cols, first):
                """Accumulate sum_k xstat.T @ W[:, k, cols] into psum_ap.
                xstat: list of stationary tiles (1 for f32r, (hi, lo) for 3pass).
                wkey: weight name base; cols: slice of the 3H axis."""
                if gru3:
                    xhi, xlo = xstat
                    whi, wlo = W[wkey + "_0"], W[wkey + "_1"]
                    passes = [(xhi, whi), (xhi, wlo), (xlo, whi)]
                else:
                    passes = [(xstat[0], W[wkey + "_0"])]
                st = first
                for (xs, ws) in passes:
                    for k in range(4):
                        nc.tensor.matmul(psum_ap, lhsT=xs[:, k, :],
                                         rhs=ws[:, k, cols],
                                         start=st, stop=False)
                        st = False

            def bias_mm(psum_ap, row):
                nc.tensor.matmul(psum_ap, lhsT=ones_t[:],
                                 rhs=bias_t[row:row + 1, :],
                                 start=False, stop=True)

            def transpose_group(raw_ap, tagbase, make_pair):
                """[32, 512] f32 -> stationary tile(s) [128, 4, 32]."""
                tr = ptr.tile([128, 128], F32, tag="ptr")
                for k in range(4):
                    nc.tensor.transpose(out=tr[:, k * B:(k + 1) * B],
                                        in_=raw_ap[:, k * 128:(k + 1) * 128],
                                        identity=ident[:B, :B])
                trv = tr[:].rearrange("p (k b) -> p k b", k=4)
                if make_pair:
                    return stat_pair(trv, tagbase)
                return (stat_one(trv, tagbase),)

            # prepare step-0 stationaries from the f32 inputs
            if gru3:
                xT = stat_pair(xT0[:], "xT")
                hT = [stat_pair(hT0_in[li][:], f"hT{li}") for li in range(2)]
            else:
                xT = (stat_one(xT0[:], "xT"),)
                hT = [(stat_one(hT0_in[li][:], f"hT{li}"),) for li in range(2)]
            hraw = [hraw_in[0], hraw_in[1]]

            if log3:
                def log_passes(h1stat):
                    hhi, hlo = h1stat
                    return [(hhi, WO[0]), (hhi, WO[1]), (hlo, WO[0])]
            else:
                def log_passes(h1stat):
                    return [(h1stat[0], WO[0])]

            # ---------------- decode loop ----------------
            for t in range(T):
                # ---- GRU layers
                new_hraw = []
                new_hT = []
                for li in range(2):
                    wx = f"w{li}x"
                    wh = f"w{li}h"
                    xs = xT if li == 0 else new_hT[0]
                    p_rz = prz.tile([B, 1024], F32, tag="prz")
                    p_n = pn.tile([B, 1024], F32, tag="pn")
                    # h parts first (independent of the incoming token) so the
                    # scheduler can hoist them into the previous step's tail,
                    # then biases, then x parts.
                    gru_matmuls(p_rz[:, 0:512], hT[li], wh, slice(0, 512), True)
                    gru_matmuls(p_rz[:, 512:1024], hT[li], wh, slice(512, 1024), True)
                    gru_matmuls(p_n[:, 512:1024], hT[li], wh, slice(1024, 1536), True)
                    bias_mm(p_n[:, 512:1024], 4 * li + 3)
                    gru_matmuls(p_rz[:, 0:512], xs, wx, slice(0, 512), False)
                    bias_mm(p_rz[:, 0:512], 4 * li + 0)
                    gru_matmuls(p_rz[:, 512:1024], xs, wx, slice(512, 1024), False)
                    bias_mm(p_rz[:, 512:1024], 4 * li + 1)
                    gru_matmuls(p_n[:, 0:512], xs, wx, slice(1024, 1536), True)
                    bias_mm(p_n[:, 0:512], 4 * li + 2)

                    # gates
                    rz = sba.tile([B, 1024], F32, tag="rz")
                    nc.scalar.activation(out=rz[:], in_=p_rz[:], func=SIG)
                    t1 = sba.tile([B, 512], F32, tag="t1")
                    nc.vector.tensor_tensor(out=t1[:], in0=rz[:, 0:512],
                                            in1=p_n[:, 512:1024], op=AOP.mult)
                    t2 = sba.tile([B, 512], F32, tag="t2")
                    nc.vector.tensor_tensor(out=t2[:], in0=t1[:],
                                            in1=p_n[:, 0:512], op=AOP.add)
                    n_sb = sba.tile([B, 512], F32, tag="n_sb")
                    nc.scalar.activation(out=n_sb[:], in_=t2[:], func=TANH)
                    dmn = sba.tile([B, 512], F32, tag="dmn")
                    nc.vector.tensor_tensor(out=dmn[:], in0=hraw[li][:],
                                            in1=n_sb[:], op=AOP.subtract)
                    zd = sba.tile([B, 512], F32, tag="zd")
                    nc.vector.tensor_tensor(out=zd[:], in0=dmn[:],
                                            in1=rz[:, 512:1024], op=AOP.mult)
                    hnew = sbp.tile([B, 512], F32, tag=f"hnew{li}")
                    nc.vector.tensor_tensor(out=hnew[:], in0=zd[:],
                                            in1=n_sb[:], op=AOP.add)
                    new_hraw.append(hnew)
                    new_hT.append(transpose_group(hnew[:], f"hT{li}", gru3))

                # ---- logits + per-chunk argmax candidates
                cv8 = sba.tile([B, NCH, 8], F32, tag="cv8")
                ci8 = sba.tile([B, NCH, 8], U32, tag="ci8")
                for j in range(NCH):
                    p_l = plog.tile([B, 512], F32, tag="plog")
                    st = True
                    for (hs, ws) in log_passes(new_hT[1]):
                        for k in range(4):
                            nc.tensor.matmul(p_l[:], lhsT=hs[:, k, :],
                                             rhs=ws[:, k, 512 * j:512 * (j + 1)],
                                             start=st, stop=False)
                            st = False
                    bias_mm(p_l[:], 8 + j)
                    nc.vector.max(out=cv8[:, j, :], in_=p_l[:])
                    nc.vector.max_index(out=ci8[:, j, :], in_max=cv8[:, j, :],
                                        in_values=p_l[:])

                # chunk-local idx -> global vocab idx (f32, exact below 2^24)
                gidx = sba.tile([B, NCH], F32, tag="gidx")
                nc.vector.tensor_copy(gidx[:], ci8[:, :, 0])
                nc.vector.tensor_tensor(out=gidx[:], in0=gidx[:],
                                        in1=chunkoff_t[:], op=AOP.add)

                # ---- exchange: AllGather the 8 chunk candidates of all cores
                cand = sba.tile([B, 2 * NCH], F32, tag="cand")
                nc.vector.tensor_copy(cand[:, 0:NCH], cv8[:, :, 0])
                nc.vector.tensor_copy(cand[:, NCH:2 * NCH], gidx[:])
                cin = dr.tile([B, 2 * NCH], F32, tag="cin")
                cout = dr.tile([NC * B, 2 * NCH], F32, tag="cout")
                nc.sync.dma_start(cin[:], cand[:])
                if nocc:
                    for cc_i in range(NC):
                        nc.sync.dma_start(cout[cc_i * B:(cc_i + 1) * B, :], cand[:])
                else:
                    nc.gpsimd.collective_compute(
                        "AllGather", AOP.bypass,
                        replica_groups=[list(range(NC))],
                        ins=[cin[:].opt()], outs=[cout[:].opt()])
                agv = sba.tile([B, NC * NCH], F32, tag="agv")
                agi = sba.tile([B, NC * NCH], F32, tag="agi")
                cc = cout[:].rearrange("(c b) t -> b c t", b=B)
                nc.sync.dma_start(agv[:], cc[:, :, 0:NCH])
                nc.sync.dma_start(agi[:], cc[:, :, NCH:2 * NCH])

                # ---- global argmax with first-occurrence tie-break
                gmax = sba.tile([B, 1], F32, tag="gmax")
                nc.vector.tensor_reduce(out=gmax[:], in_=agv[:], op=AOP.max,
                                        axis=mybir.AxisListType.X)
                mask = sba.tile([B, NC * NCH], F32, tag="mask")
                nc.vector.tensor_scalar(mask[:], agv[:], gmax[:, 0:1], None,
                                        op0=AOP.is_ge)
                msel = sba.tile([B, NC * NCH], F32, tag="msel")
                nc.vector.tensor_scalar(msel[:], agi[:], 1e7, None,
                                        op0=AOP.subtract)
                nc.vector.tensor_tensor(out=msel[:], in0=msel[:], in1=mask[:],
                                        op=AOP.mult)
                nc.vector.tensor_scalar(msel[:], msel[:], 1e7, None, op0=AOP.add)
                tok_f = sba.tile([B, 1], F32, tag="tok_f")
                nc.vector.tensor_reduce(out=tok_f[:], in_=msel[:], op=AOP.min,
                                        axis=mybir.AxisListType.XY)
                tok_i = sbp.tile([B, 1], I32, tag="tok_i")
                nc.vector.tensor_copy(tok_i[:], tok_f[:])
                nc.vector.tensor_copy(toks_sb[:, t:t + 1], tok_i[:])

                # ---- next x: gather + transpose (skip after last step)
                hraw = new_hraw
                hT = new_hT
                if t + 1 < T:
                    x_raw = sba.tile([B, E], F32, tag="x_raw")
                    nc.gpsimd.indirect_dma_start(
                        out=x_raw[:], out_offset=None, in_=d["emb"][:],
                        in_offset=bass.IndirectOffsetOnAxis(ap=tok_i[:, 0:1], axis=0))
                    xT = transpose_group(x_raw[:], "xT", gru3)

            # ---------------- outputs ----------------
            nc.sync.dma_start(out_toks[:], toks_sb[:])
            nc.sync.dma_start(out_h[0], hraw[0][:])
            nc.sync.dma_start(out_h[1], hraw[1][:])

    nc.compile()
    return nc


def _prep_inputs(h_0, emb, W_ih0, W_hh0, b_ih0, b_hh0, W_ih1, W_hh1, b_ih1,
                 b_hh1, W_out, b_out, gru3, log3):
    SOS = 1
    rnd = (lambda a: a) if True else None
    gw = {}
    for li, (wx, wh) in enumerate([(W_ih0, W_hh0), (W_ih1, W_hh1)]):
        for part, w in (("x", wx), ("h", wh)):
            kT = _to_kT(w)          # [128, 4, 1536]
            if gru3:
                hi, lo = _hi_lo(kT)
                gw[f"w{li}{part}_0"] = hi
                gw[f"w{li}{part}_1"] = lo
            else:
                gw[f"w{li}{part}_0"] = _round_tf32(kT)

    x0 = np.repeat(emb[SOS][None, :], B, axis=0)        # [32, 512]
    common = dict(
        emb=np.ascontiguousarray(emb),
        x0T=np.ascontiguousarray(x0.T.reshape(4, 128, B).transpose(1, 0, 2)),
        h0T=np.ascontiguousarray(h_0[0].T.reshape(4, 128, B).transpose(1, 0, 2)),
        h1T=np.ascontiguousarray(h_0[1].T.reshape(4, 128, B).transpose(1, 0, 2)),
        h0raw=np.ascontiguousarray(h_0[0]),
        h1raw=np.ascontiguousarray(h_0[1]),
        **gw,
    )

    brz0 = (b_ih0 + b_hh0)[0:1024]
    brz1 = (b_ih1 + b_hh1)[0:1024]
    in_maps = []
    for c in range(NC):
        wsh = W_out[c * VS:(c + 1) * VS]
        wpad = np.zeros((VSP, H), np.float32)
        wpad[:VS] = wsh
        kT = _to_kT(wpad)                                # [128, 4, 4096]
        wo = {}
        if log3:
            hi, lo = _hi_lo(kT)
            wo["wout_0"], wo["wout_1"] = hi, lo
        else:
            wo["wout_0"] = _round_tf32(kT)
        bpad = np.full((VSP,), -1e30, np.float32)
        bpad[:VS] = b_out[c * VS:(c + 1) * VS]
        bias_all = np.zeros((16, 512), np.float32)
        bias_all[0:2] = brz0.reshape(2, 512)
        bias_all[2] = b_ih0[1024:1536]
        bias_all[3] = b_hh0[1024:1536]
        bias_all[4:6] = brz1.reshape(2, 512)
        bias_all[6] = b_ih1[1024:1536]
        bias_all[7] = b_hh1[1024:1536]
        bias_all[8:16] = bpad.reshape(8, 512)
        chunkoff = np.zeros((B, NCH), np.float32)
        chunkoff[:] = (c * VS + 512 * np.arange(NCH))[None, :]
        in_maps.append(dict(common, bias_all=_round_tf32(bias_all),
                            chunkoff=chunkoff, **wo))
    return in_maps


def kernel(h_0, emb, W_ih0, W_hh0, b_ih0, b_hh0, W_ih1, W_hh1, b_ih1, b_hh1,
           W_out, b_out, max_n, _trace=False):
    T = int(max_n)
    gru3 = MODE in ("3pass", "gru3pass")
    log3 = MODE == "3pass"
    args = tuple(np.asarray(a) for a in
                 (h_0, emb, W_ih0, W_hh0, b_ih0, b_hh0, W_ih1, W_hh1,
                  b_ih1, b_hh1, W_out, b_out))
    in_maps = _prep_inputs(*[np.asarray(a, np.float32) for a in args],
                           gru3=gru3, log3=log3)
    key = (T, gru3, log3)
    if key not in _cache:
        _cache[key] = build(T, gru3, log3)
    nc = _cache[key]
    res = run_bass_kernel_spmd(nc, in_maps, core_ids=list(range(NC)),
                               trace=_trace)
    r0 = res.results[0]
    toks = r0["out_toks"].astype(np.int32)
    h_t = r0["out_h"].astype(np.float32)
    if _trace:
        kernel.last_exec_ns = res.exec_time_ns
    return toks, h_t
